# revision 13
# baseline (speedup 1.0000x reference)
"""Trainium2 Bass kernel for the autoregressive LSTM decoder.

Problem: B=64, T=512 decoder steps, latent L=256, hidden H=1024.
tf_prob=0 and the per-step uniform draws (key 42) are all > 0, so the
decoder is purely autoregressive: targets is never used and the input
matmul folds into the hidden matmul:

    x_{t+1} = out_t = h_t @ w_fc.T + b_fc
    gates_{t+1} = h_t @ (w_fc.T @ w_ih.T + w_hh.T) + (b + b_fc @ w_ih.T)
                = h_t @ W_eff + b_eff

Step 0 (initial_input / h0 / c0) is computed on the host in fp32; the
device runs steps 1..511 of the collapsed recurrence.

v2 design (HW-measured: MMs serialize on PE at ~211-290 ns per
[128,64]x[128,512] bf16 matmul; no col-group concurrency):
  - Gate chunk pair -> ONE merged PSUM bank [128, 512]: even chunk at
    partitions 0:64 (tile_position (0,0)), odd at 64:128 ((0,64)),
    interleaved. Measured 211 ns/MM vs 269-294 for split-bank variants.
  - Gate bias added on DVE (PSUM in-place, [128,512] broadcast tile),
    not on PE: saves 8 bias matmuls/step.
  - fc projection deferred out of the loop entirely: h_T stored to HBM
    (bf16) each step; a final GEMM sharded over the 8 cores by time
    (partition_id) computes all outputs. Saves ~9 matmuls/step.
  - Nonlin runs as full-128-partition ACT/DVE instructions.
  - Transpose (PE) of each pair is emitted AFTER the next pair's matmul
    block ("rotation") so the PE never idles waiting for the ACT/DVE
    nonlinearity chain; the per-step tail transpose is deferred into the
    next step's first matmul block.

The time recurrence is strictly sequential; per-step cross-core
collectives have a ~7-20us floor, so all 8 cores run the same recurrence
(replicated) and cooperate only on the final time-sharded GEMM.
"""

import os
import numpy as np

B, T, L, H = 64, 512, 256, 1024
P = 128
NK = H // P            # 8 k-tiles
G4 = 4 * H             # 4096 gate cols
NCH = NK               # 8 gate chunks of 512 cols
NPAIR = NCH // 2       # 4 chunk pairs

_prog_cache = {}


def _gate_perm():
    """Column permutation of [4H] gate space -> per-chunk [i|f|o|g]."""
    perm = np.empty(G4, np.int64)
    pos = 0
    for k in range(NK):
        for gidx in (0, 1, 3, 2):   # i, f, o, g  (torch order i,f,g,o)
            base = gidx * H + k * P
            perm[pos:pos + P] = np.arange(base, base + P)
            pos += P
    return perm


def _build_program(mm_dt_name: str = "bf16", n_steps: int = T):
    import concourse.bass as bass
    import concourse.bacc as bacc
    import concourse.mybir as mybir
    from concourse.bass import ts
    from concourse.tile import TileContext

    f32 = mybir.dt.float32
    bf16 = mybir.dt.bfloat16
    split_g = os.environ.get("BASS_LSTM_SPLITG", "0") == "1"
    g1_start = os.environ.get("BASS_LSTM_G1START", "1") == "1"
    workbufs = int(os.environ.get("BASS_LSTM_WORKBUFS", "4"))
    psumg_bufs = int(os.environ.get("BASS_LSTM_PSUMG_BUFS", "3"))
    bias_mode = os.environ.get("BASS_LSTM_BIAS", "dve")  # dve | mm
    no_hstore = os.environ.get("BASS_LSTM_NOHSTORE", "0") == "1"
    unroll = int(os.environ.get("BASS_LSTM_UNROLL", "32"))

    nc = bacc.Bacc(None, target_bir_lowering=False)
    AF = mybir.ActivationFunctionType

    # ---- DRAM I/O ----
    hT0 = nc.declare_dram_parameter("hT0", [H, B], bf16, isOutput=False)
    c0 = nc.declare_dram_parameter("c0", [P, H // 2], f32, isOutput=False)
    Wg = nc.declare_dram_parameter("Wg", [H, G4], bf16, isOutput=False)
    # bias, pair-packed + replicated: [128, NPAIR*512]; col block cq rows
    # 0:64 = even-chunk bias, rows 64:128 = odd-chunk bias
    biasF_d = nc.declare_dram_parameter("biasF", [P, NPAIR * 512], f32,
                                        isOutput=False)
    # K=2 bias matmul operands: sel2 [2,128] (row0 -> parts 0:64, row1 ->
    # 64:128), bg2 [2, NPAIR*512] (row0 even-chunk bias, row1 odd)
    sel2_d = nc.declare_dram_parameter("sel2", [2, P], bf16, isOutput=False)
    bg2_d = nc.declare_dram_parameter("bg2", [2, NPAIR * 512], bf16,
                                      isOutput=False)
    Wf = nc.declare_dram_parameter("Wf", [H, L], bf16, isOutput=False)
    bfc_d = nc.declare_dram_parameter("bfc", [P, 2], f32, isOutput=False)
    ident_d = nc.declare_dram_parameter("ident128", [P, P], bf16, isOutput=False)
    # h_T history for the deferred fc GEMM: rows = h-dim, cols = t*B + b
    hT_store = nc.dram_tensor("hT_store", [H, n_steps * B], bf16,
                              kind="Internal")
    # out.T: rows = L, cols = t*B + b  (out for step t+1 in col block t)
    outsT = nc.declare_dram_parameter("outsT", [L, n_steps * B], f32,
                                      isOutput=True)

    n_quads = n_steps // 4
    assert n_steps % 4 == 0

    with TileContext(nc) as tc:
        with (
            tc.tile_pool(name="consts", bufs=1) as consts,
            tc.tile_pool(name="state", bufs=1) as state,
            tc.tile_pool(name="work", bufs=workbufs) as work,
            tc.tile_pool(name="rpool", bufs=2) as rpool,
            tc.tile_pool(name="psumG", bufs=psumg_bufs, space="PSUM") as psumG,
            tc.tile_pool(name="psumS", bufs=2, space="PSUM") as psumS,
        ):
            # ---- constants / weights resident in SBUF ----
            W_sb = consts.tile([P, NK * G4], bf16, tag="W")
            for k in range(NK):
                nc.sync.dma_start(out=W_sb[:, k * G4:(k + 1) * G4],
                                  in_=Wg[k * P:(k + 1) * P, :])
            biasF = consts.tile([P, NPAIR * 512], f32, tag="biasF")
            nc.sync.dma_start(out=biasF[:], in_=biasF_d[:])
            sel2 = consts.tile([2, P], bf16, tag="sel2")
            nc.sync.dma_start(out=sel2[:], in_=sel2_d[:])
            bg2 = consts.tile([2, NPAIR * 512], bf16, tag="bg2")
            nc.sync.dma_start(out=bg2[:], in_=bg2_d[:])
            Wf_sb = consts.tile([P, NK * L], bf16, tag="Wf")
            for k in range(NK):
                nc.sync.dma_start(out=Wf_sb[:, k * L:(k + 1) * L],
                                  in_=Wf[k * P:(k + 1) * P, :])
            bfc_sb = consts.tile([P, 2], f32, tag="bfc")
            nc.sync.dma_start(out=bfc_sb[:], in_=bfc_d[:])
            ident128 = consts.tile([P, P], bf16, tag="ident128")
            nc.sync.dma_start(out=ident128[:], in_=ident_d[:])

            # h kept as 8 separate per-chunk tiles (cross-step deps per chunk)
            hA = [state.tile([P, B], bf16, tag=f"hA{k}", name=f"hA{k}")
                  for k in range(NK)]
            hB = [state.tile([P, B], bf16, tag=f"hB{k}", name=f"hB{k}")
                  for k in range(NK)]
            # c packed: tile cq holds even chunk at partitions 0:64, odd at
            # 64:128 (cols = 128 h-cols of that chunk)
            c_tiles = [state.tile([P, P], f32, tag=f"c{q}", name=f"c{q}")
                       for q in range(NPAIR)]
            for k in range(NK):
                nc.sync.dma_start(out=hA[k][:], in_=hT0[k * P:(k + 1) * P, :])
            for q, ct in enumerate(c_tiles):
                nc.sync.dma_start(out=ct[:], in_=c0[:, q * P:(q + 1) * P])

            def mm_pair(G, hsrc, cq, ks):
                """Accumulate gate chunk pair cq over k-tiles `ks` into the
                merged PSUM tile G [128, 512]."""
                ce, co = 2 * cq, 2 * cq + 1
                last = (NK - 1) if bias_mode != "mm" else -1
                for k in ks:
                    nc.tensor.matmul(
                        G[0:B, :], lhsT=hsrc[k][:],
                        rhs=W_sb[:, k * G4 + ce * 512: k * G4 + ce * 512 + 512],
                        start=(k == 0), stop=(k == last),
                        tile_position=(0, 0), skip_group_check=True,
                    )
                    nc.tensor.matmul(
                        G[B:P, :], lhsT=hsrc[k][:],
                        rhs=W_sb[:, k * G4 + co * 512: k * G4 + co * 512 + 512],
                        start=(k == 0 and g1_start), stop=(k == last),
                        tile_position=(0, B), skip_group_check=True,
                    )
                if bias_mode == "mm" and (NK - 1) in ks:
                    nc.tensor.matmul(
                        G[0:P, :], lhsT=sel2[:],
                        rhs=bg2[:, cq * 512:(cq + 1) * 512],
                        start=False, stop=True, skip_group_check=True,
                    )

            def mm_pair_split(G0, G1, hsrc, cq, ks):
                ce, co = 2 * cq, 2 * cq + 1
                for k in ks:
                    nc.tensor.matmul(
                        G0[0:B, :], lhsT=hsrc[k][:],
                        rhs=W_sb[:, k * G4 + ce * 512: k * G4 + ce * 512 + 512],
                        start=(k == 0), stop=(k == NK - 1),
                        tile_position=(0, 0),
                    )
                    nc.tensor.matmul(
                        G1[B:P, :], lhsT=hsrc[k][:],
                        rhs=W_sb[:, k * G4 + co * 512: k * G4 + co * 512 + 512],
                        start=(k == 0), stop=(k == NK - 1),
                        tile_position=(0, B),
                    )

            def nl_merged(G, cq):
                """Bias + nonlinearities for pair cq, 128-partition ops.
                Returns the batch-major h pair tile (bf16)."""
                if bias_mode != "mm":
                    nc.vector.tensor_add(G[:, :], G[:, :],
                                         biasF[:, cq * 512:(cq + 1) * 512])
                sig = work.tile([P, 384], f32, tag="sig")
                nc.scalar.activation(sig[:], G[:, 0:384], AF.Sigmoid)
                tg = work.tile([P, P], f32, tag="tg")
                nc.scalar.activation(tg[:], G[:, 384:512], AF.Tanh)
                ct = c_tiles[cq]
                t1 = work.tile([P, P], f32, tag="t1")
                nc.vector.tensor_mul(t1[:], sig[:, 0:P], tg[:])
                t2 = work.tile([P, P], f32, tag="t2")
                nc.vector.tensor_mul(t2[:], sig[:, P:2 * P], ct[:])
                nc.vector.tensor_add(ct[:], t1[:], t2[:])
                tc2 = work.tile([P, P], f32, tag="tc2")
                nc.scalar.activation(tc2[:], ct[:], AF.Tanh)
                hbm2 = work.tile([P, P], bf16, tag="hbm")
                nc.vector.tensor_mul(hbm2[:], sig[:, 2 * P:3 * P], tc2[:])
                return hbm2

            def nl_split(G0, G1, cq):
                nc.vector.tensor_add(G0[0:B, :], G0[0:B, :],
                                     biasF[0:B, cq * 512:(cq + 1) * 512])
                nc.vector.tensor_add(G1[B:P, :], G1[B:P, :],
                                     biasF[B:P, cq * 512:(cq + 1) * 512])
                sig = work.tile([P, 384], f32, tag="sig")
                nc.scalar.activation(sig[0:B, :], G0[0:B, 0:384], AF.Sigmoid)
                nc.scalar.activation(sig[B:P, :], G1[B:P, 0:384], AF.Sigmoid)
                tg = work.tile([P, P], f32, tag="tg")
                nc.scalar.activation(tg[0:B, :], G0[0:B, 384:512], AF.Tanh)
                nc.scalar.activation(tg[B:P, :], G1[B:P, 384:512], AF.Tanh)
                ct = c_tiles[cq]
                t1 = work.tile([P, P], f32, tag="t1")
                nc.vector.tensor_mul(t1[:], sig[:, 0:P], tg[:])
                t2 = work.tile([P, P], f32, tag="t2")
                nc.vector.tensor_mul(t2[:], sig[:, P:2 * P], ct[:])
                nc.vector.tensor_add(ct[:], t1[:], t2[:])
                tc2 = work.tile([P, P], f32, tag="tc2")
                nc.scalar.activation(tc2[:], ct[:], AF.Tanh)
                hbm2 = work.tile([P, P], bf16, tag="hbm")
                nc.vector.tensor_mul(hbm2[:], sig[:, 2 * P:3 * P], tc2[:])
                return hbm2

            def tr(hbm2, hdst, cq, t_expr):
                """PE transpose of the pair's batch-major h; writes the two
                h_T chunks and streams them to the HBM history."""
                ce, co = 2 * cq, 2 * cq + 1
                pt = psumS.tile([P, P], bf16, tag="pt")
                nc.tensor.transpose(pt[:], hbm2[:], ident128[:])
                nc.vector.tensor_copy(hdst[ce][:], pt[:, 0:B])
                nc.vector.tensor_copy(hdst[co][:], pt[:, B:P])
                if not no_hstore:
                    nc.sync.dma_start(
                        out=hT_store[ce * P:(ce + 1) * P, ts(t_expr, B)],
                        in_=hdst[ce][:])
                    nc.sync.dma_start(
                        out=hT_store[co * P:(co + 1) * P, ts(t_expr, B)],
                        in_=hdst[co][:])

            def step(hsrc, hdst, t_expr, pend):
                """One decoder step; `pend` is the previous step's pair-3
                (hbm2, hdst, cq, t) transpose, emitted inside this step's
                first matmul block. Returns this step's pair-3 pend."""
                pends = [None] * NPAIR
                for cq in range(NPAIR):
                    if split_g:
                        Ga = psumG.tile([P, 512], f32, tag="Ga")
                        Gb = psumG.tile([P, 512], f32, tag="Gb")
                        if cq == 0 and pend is not None:
                            mm_pair_split(Ga, Gb, hsrc, cq, range(0, 6))
                            tr(*pend)
                            mm_pair_split(Ga, Gb, hsrc, cq, range(6, 8))
                        else:
                            mm_pair_split(Ga, Gb, hsrc, cq, range(NK))
                        if cq > 0:
                            tr(*pends[cq - 1])
                        pends[cq] = (nl_split(Ga, Gb, cq), hdst, cq, t_expr)
                    else:
                        G = psumG.tile([P, 512], f32, tag="G")
                        if cq == 0 and pend is not None:
                            mm_pair(G, hsrc, cq, range(0, 6))
                            tr(*pend)
                            mm_pair(G, hsrc, cq, range(6, 8))
                        else:
                            mm_pair(G, hsrc, cq, range(NK))
                        if cq > 0:
                            tr(*pends[cq - 1])
                        pends[cq] = (nl_merged(G, cq), hdst, cq, t_expr)
                return pends[NPAIR - 1]

            assert n_steps % unroll == 0 and unroll % 2 == 0
            with tc.For_i(0, n_steps // unroll, staggered_reset=True) as i:
                pend = None
                for j in range(unroll):
                    src, dst = (hA, hB) if j % 2 == 0 else (hB, hA)
                    pend = step(src, dst, i * unroll + j, pend)
                tr(*pend)

            # ---- deferred fc: out.T = Wf.T @ h_T + b_fc, T-sharded ----
            # Each core handles n_steps/8 steps = NK col-chunks of 512.
            pid_sp = nc.sync.partition_id()
            pid_act = nc.scalar.partition_id()
            nch_per_core = (n_steps * B) // 512 // 8
            for nch in range(nch_per_core):
                rhs = rpool.tile([P, NK * 512], bf16, tag="rhs")
                for k in range(NK):
                    nc.sync.dma_start(
                        out=rhs[:, k * 512:(k + 1) * 512],
                        in_=hT_store[k * P:(k + 1) * P,
                                     ts(pid_sp * nch_per_core + nch, 512)],
                    )
                for m in range(2):
                    O = psumS.tile([P, 512], f32, tag="O")
                    for k in range(NK):
                        nc.tensor.matmul(
                            O[:], lhsT=Wf_sb[:, k * L + m * P: k * L + (m + 1) * P],
                            rhs=rhs[:, k * 512:(k + 1) * 512],
                            start=(k == 0), stop=(k == NK - 1),
                        )
                    osb = work.tile([P, 512], f32, tag="osb")
                    nc.scalar.activation(osb[:], O[:], AF.Identity,
                                         bias=bfc_sb[:, m:m + 1])
                    nc.scalar.dma_start(
                        out=outsT[m * P:(m + 1) * P,
                                  ts(pid_act * nch_per_core + nch, 512)],
                        in_=osb[:])

    if not nc.is_finalized():
        nc.finalize()
    return nc


def _prepare_host_inputs(initial_input, h0, c0, w_ih, w_hh, b_ih, b_hh,
                         w_fc, b_fc, mm_dt_name="bf16"):
    """Host: fp32 step 0 + collapsed weights, permuted for the device."""
    import ml_dtypes

    f64 = np.float64
    w_ih64, w_hh64 = w_ih.astype(f64), w_hh.astype(f64)
    w_fc64, b_fc64 = w_fc.astype(f64), b_fc.astype(f64)
    bias64 = b_ih.astype(f64) + b_hh.astype(f64)

    W_eff = (w_fc64.T @ w_ih64.T + w_hh64.T).astype(np.float32)   # [H, 4H]
    b_eff = (bias64 + b_fc64 @ w_ih64.T).astype(np.float32)       # [4H]

    def sigmoid(x):
        return 1.0 / (1.0 + np.exp(-x))

    x = initial_input.astype(np.float32)
    h = h0[0].astype(np.float32)
    c = c0[0].astype(np.float32)
    g = x @ w_ih.T.astype(np.float32) + h @ w_hh.T.astype(np.float32) \
        + (bias64.astype(np.float32))
    i_, f_, g_, o_ = np.split(g, 4, axis=1)
    c = sigmoid(f_) * c + sigmoid(i_) * np.tanh(g_)
    h = sigmoid(o_) * np.tanh(c)
    out0 = h @ w_fc.T.astype(np.float32) + b_fc.astype(np.float32)

    perm = _gate_perm()
    Wg = np.ascontiguousarray(W_eff[:, perm])
    bg = np.ascontiguousarray(b_eff[perm])            # [4096] permuted
    Wf = np.ascontiguousarray(w_fc.T.astype(np.float32))
    hT = np.ascontiguousarray(h.T)

    bf16 = ml_dtypes.bfloat16

    # biasF [128, NPAIR*512]: block cq rows 0:64 even-chunk bias, 64:128 odd
    biasF = np.empty((P, NPAIR * 512), np.float32)
    for cq in range(NPAIR):
        ce, co = 2 * cq, 2 * cq + 1
        biasF[0:B, cq * 512:(cq + 1) * 512] = bg[ce * 512:(ce + 1) * 512]
        biasF[B:P, cq * 512:(cq + 1) * 512] = bg[co * 512:(co + 1) * 512]

    # c packed [64, 1024] -> [128, 512]: chunk k at partitions (k%2)*64,
    # cols (k//2)*128
    c_packed = np.zeros((P, H // 2), np.float32)
    for k in range(NK):
        c_packed[(k % 2) * B:(k % 2) * B + B,
                 (k // 2) * P:(k // 2) * P + P] = c[:, k * P:(k + 1) * P]

    bfc = np.stack([b_fc[0:P], b_fc[P:2 * P]], axis=1).astype(np.float32)

    sel2 = np.zeros((2, P), np.float32)
    sel2[0, 0:B] = 1.0
    sel2[1, B:P] = 1.0
    bg2 = np.empty((2, NPAIR * 512), np.float32)
    for cq in range(NPAIR):
        bg2[0, cq * 512:(cq + 1) * 512] = bg[(2 * cq) * 512:(2 * cq + 1) * 512]
        bg2[1, cq * 512:(cq + 1) * 512] = bg[(2 * cq + 1) * 512:(2 * cq + 2) * 512]

    in_map = {
        "hT0": hT.astype(bf16),
        "c0": np.ascontiguousarray(c_packed),
        "Wg": Wg.astype(bf16),
        "biasF": biasF,
        "sel2": sel2.astype(bf16),
        "bg2": bg2.astype(bf16),
        "Wf": Wf.astype(bf16),
        "bfc": np.ascontiguousarray(bfc),
        "ident128": np.eye(P, dtype=np.float32).astype(bf16),
    }
    return in_map, out0


LAST_EXEC_NS = None

# min over jax.random.uniform(jax.random.key(42), (512,)) — the per-step
# teacher-forcing draws inside the reference. tf_prob below this means the
# decoder is purely autoregressive (the case the device kernel implements).
_RAND_MIN = 5.8138370513916016e-04


def _kernel_numpy_fallback(initial_input, h0, c0, targets, tf_prob,
                           w_ih, w_hh, b_ih, b_hh, w_fc, b_fc):
    """Host fp32 implementation incl. teacher forcing (only used if
    tf_prob >= min(rand), which the problem spec never produces)."""
    import jax
    import jax.numpy as jnp
    cpu = jax.devices("cpu")[0]
    with jax.default_device(cpu):
        rand = np.asarray(
            jax.random.uniform(jax.random.key(42), (T,), jnp.float32))

    def sigmoid(x):
        return 1.0 / (1.0 + np.exp(-x))

    bias = (b_ih + b_hh).astype(np.float32)
    h = h0[0].astype(np.float32)
    c = c0[0].astype(np.float32)
    inp = initial_input.astype(np.float32)
    outs = []
    for t in range(T):
        g = inp @ w_ih.T + h @ w_hh.T + bias
        i, f, gg, o = np.split(g, 4, axis=1)
        c = sigmoid(f) * c + sigmoid(i) * np.tanh(gg)
        h = sigmoid(o) * np.tanh(c)
        out = h @ w_fc.T + b_fc
        inp = out if rand[t] > tf_prob else targets[:, t, :]
        outs.append(out)
    return np.stack(outs, axis=1)[:, None, :, :].astype(np.float32)


def _assemble_output(results, out0, n_cores=8):
    """Assemble [B, 1, T, L] from per-core outsT slices."""
    nch_per_core = (T * B) // 512 // n_cores
    cols_per_core = nch_per_core * 512
    outT = np.empty((L, T * B), np.float32)
    for c in range(n_cores):
        sl = slice(c * cols_per_core, (c + 1) * cols_per_core)
        outT[:, sl] = results[c]["outsT"][:, sl]
    # col t*B+b holds out for step t+1; steps 1..511 used
    o = outT.reshape(L, T, B)
    out = np.empty((B, 1, T, L), np.float32)
    out[:, 0, 0, :] = out0
    out[:, 0, 1:, :] = np.transpose(o[:, :T - 1, :], (2, 1, 0))
    return out


def kernel(initial_input, encoder_outputs, h0, c0, targets, tf_prob,
           w_ih, w_hh, b_ih, b_hh, w_fc, b_fc):
    global LAST_EXEC_NS
    from concourse.bass_utils import run_bass_kernel_spmd

    if float(np.asarray(tf_prob)) >= _RAND_MIN:
        return _kernel_numpy_fallback(
            np.asarray(initial_input), np.asarray(h0), np.asarray(c0),
            np.asarray(targets), float(np.asarray(tf_prob)),
            np.asarray(w_ih), np.asarray(w_hh), np.asarray(b_ih),
            np.asarray(b_hh), np.asarray(w_fc), np.asarray(b_fc))

    mm_dt_name = "bf16"
    n_cores = 8

    if mm_dt_name not in _prog_cache:
        _prog_cache[mm_dt_name] = _build_program(mm_dt_name)
    nc = _prog_cache[mm_dt_name]

    in_map, out0 = _prepare_host_inputs(
        np.asarray(initial_input), np.asarray(h0), np.asarray(c0),
        np.asarray(w_ih), np.asarray(w_hh), np.asarray(b_ih),
        np.asarray(b_hh), np.asarray(w_fc), np.asarray(b_fc), mm_dt_name,
    )

    core_ids = list(range(n_cores))
    res = run_bass_kernel_spmd(nc, [in_map] * n_cores, core_ids=core_ids)
    LAST_EXEC_NS = res.exec_time_ns
    return _assemble_output(res.results, out0, n_cores)


# revision 44
# speedup vs baseline: 1.2970x; 1.2970x over previous
"""Trainium2 Bass kernel for the autoregressive LSTM decoder.

Problem: B=64, T=512 decoder steps, latent L=256, hidden H=1024.
tf_prob=0 and the per-step uniform draws (key 42) are all > 0, so the
decoder is purely autoregressive: targets is never used and the input
matmul folds into the hidden matmul:

    x_{t+1} = out_t = h_t @ w_fc.T + b_fc
    gates_{t+1} = h_t @ (w_fc.T @ w_ih.T + w_hh.T) + (b + b_fc @ w_ih.T)
                = h_t @ W_eff + b_eff

Step 0 (initial_input / h0 / c0) is computed on the host in fp32; the
device runs steps 1..511 of the collapsed recurrence.

Default path ("fp8dr", _build_program_fp8): fp8e4m3 DoubleRow gate
matmuls -- 2 fp8 weights per PE cell, K=256 per matmul, so 32 gate
matmuls/step instead of 64 (HW-measured: matmuls serialize on the PE;
col-group concurrency does not materialize).  The 512-step recurrence
in fp8 h/W lands at rel err ~7e-3 (tolerance 2e-2); the h history and
final fc GEMM stay bf16.  DoubleRow outputs are ISA-pinned to PSUM
partitions 0:64 (walrus s3d3_mm_valid_dst_partition), so the
nonlinearity runs per-chunk at 64-partition width; stages are
software-pipelined across chunk blocks (sigmoids -> c-update ->
tanh(c) -> h-mul -> transpose, each lagging its producer by a block) so
neither the ACT nor DVE FIFO waits on a cross-engine round-trip.  The
per-chunk bias matmul (h-independent) is issued first in each block to
cover the previous step's tail chain.  fc is deferred entirely: h_T
streams to HBM each step and a final GEMM sharded over the 8 cores by
time (partition_id) produces out.T.  A bf16 merged-bank path
("bf16", _build_program) is kept as fallback.

The time recurrence is strictly sequential; per-step cross-core
collectives have a ~7-20us floor, so all 8 cores run the same recurrence
(replicated) and cooperate only on the final time-sharded GEMM.
"""

import os
import numpy as np

B, T, L, H = 64, 512, 256, 1024
P = 128
NK = H // P            # 8 k-tiles
G4 = 4 * H             # 4096 gate cols
NCH = NK               # 8 gate chunks of 512 cols
NPAIR = NCH // 2       # 4 chunk pairs

_prog_cache = {}


def _gate_perm():
    """Column permutation of [4H] gate space -> per-chunk [i|f|o|g]."""
    perm = np.empty(G4, np.int64)
    pos = 0
    for k in range(NK):
        for gidx in (0, 1, 3, 2):   # i, f, o, g  (torch order i,f,g,o)
            base = gidx * H + k * P
            perm[pos:pos + P] = np.arange(base, base + P)
            pos += P
    return perm


def _build_program(mm_dt_name: str = "bf16", n_steps: int = T):
    if mm_dt_name == "fp8dr":
        return _build_program_fp8(mm_dt_name, n_steps)
    if mm_dt_name == "fp8p":
        return _build_program_fp8p(mm_dt_name, n_steps)
    import concourse.bass as bass
    import concourse.bacc as bacc
    import concourse.mybir as mybir
    from concourse.bass import ts
    from concourse.tile import TileContext

    f32 = mybir.dt.float32
    bf16 = mybir.dt.bfloat16
    split_g = os.environ.get("BASS_LSTM_SPLITG", "0") == "1"
    g1_start = os.environ.get("BASS_LSTM_G1START", "1") == "1"
    workbufs = int(os.environ.get("BASS_LSTM_WORKBUFS", "4"))
    psumg_bufs = int(os.environ.get("BASS_LSTM_PSUMG_BUFS", "3"))
    bias_mode = os.environ.get("BASS_LSTM_BIAS", "dve")  # dve | mm
    no_hstore = os.environ.get("BASS_LSTM_NOHSTORE", "0") == "1"
    unroll = int(os.environ.get("BASS_LSTM_UNROLL", "32"))

    nc = bacc.Bacc(None, target_bir_lowering=False)
    AF = mybir.ActivationFunctionType

    # ---- DRAM I/O ----
    hT0 = nc.declare_dram_parameter("hT0", [H, B], bf16, isOutput=False)
    c0 = nc.declare_dram_parameter("c0", [P, H // 2], f32, isOutput=False)
    Wg = nc.declare_dram_parameter("Wg", [H, G4], bf16, isOutput=False)
    # bias, pair-packed + replicated: [128, NPAIR*512]; col block cq rows
    # 0:64 = even-chunk bias, rows 64:128 = odd-chunk bias
    biasF_d = nc.declare_dram_parameter("biasF", [P, NPAIR * 512], f32,
                                        isOutput=False)
    # K=2 bias matmul operands: sel2 [2,128] (row0 -> parts 0:64, row1 ->
    # 64:128), bg2 [2, NPAIR*512] (row0 even-chunk bias, row1 odd)
    sel2_d = nc.declare_dram_parameter("sel2", [2, P], bf16, isOutput=False)
    bg2_d = nc.declare_dram_parameter("bg2", [2, NPAIR * 512], bf16,
                                      isOutput=False)
    Wf = nc.declare_dram_parameter("Wf", [H, L], bf16, isOutput=False)
    bfc_d = nc.declare_dram_parameter("bfc", [P, 2], f32, isOutput=False)
    ident_d = nc.declare_dram_parameter("ident128", [P, P], bf16, isOutput=False)
    # h_T history for the deferred fc GEMM: rows = h-dim, cols = t*B + b
    hT_store = nc.dram_tensor("hT_store", [H, n_steps * B], bf16,
                              kind="Internal")
    # out.T: rows = L, cols = t*B + b  (out for step t+1 in col block t)
    outsT = nc.declare_dram_parameter("outsT", [L, n_steps * B], f32,
                                      isOutput=True)

    n_quads = n_steps // 4
    assert n_steps % 4 == 0

    with TileContext(nc) as tc:
        with (
            tc.tile_pool(name="consts", bufs=1) as consts,
            tc.tile_pool(name="state", bufs=1) as state,
            tc.tile_pool(name="work", bufs=workbufs) as work,
            tc.tile_pool(name="rpool", bufs=2) as rpool,
            tc.tile_pool(name="psumG", bufs=psumg_bufs, space="PSUM") as psumG,
            tc.tile_pool(name="psumS", bufs=2, space="PSUM") as psumS,
        ):
            # ---- constants / weights resident in SBUF ----
            W_sb = consts.tile([P, NK * G4], bf16, tag="W")
            for k in range(NK):
                nc.sync.dma_start(out=W_sb[:, k * G4:(k + 1) * G4],
                                  in_=Wg[k * P:(k + 1) * P, :])
            biasF = consts.tile([P, NPAIR * 512], f32, tag="biasF")
            nc.sync.dma_start(out=biasF[:], in_=biasF_d[:])
            sel2 = consts.tile([2, P], bf16, tag="sel2")
            nc.sync.dma_start(out=sel2[:], in_=sel2_d[:])
            bg2 = consts.tile([2, NPAIR * 512], bf16, tag="bg2")
            nc.sync.dma_start(out=bg2[:], in_=bg2_d[:])
            Wf_sb = consts.tile([P, NK * L], bf16, tag="Wf")
            for k in range(NK):
                nc.sync.dma_start(out=Wf_sb[:, k * L:(k + 1) * L],
                                  in_=Wf[k * P:(k + 1) * P, :])
            bfc_sb = consts.tile([P, 2], f32, tag="bfc")
            nc.sync.dma_start(out=bfc_sb[:], in_=bfc_d[:])
            ident128 = consts.tile([P, P], bf16, tag="ident128")
            nc.sync.dma_start(out=ident128[:], in_=ident_d[:])

            # h kept as 8 separate per-chunk tiles (cross-step deps per chunk)
            hA = [state.tile([P, B], bf16, tag=f"hA{k}", name=f"hA{k}")
                  for k in range(NK)]
            hB = [state.tile([P, B], bf16, tag=f"hB{k}", name=f"hB{k}")
                  for k in range(NK)]
            # c packed: tile cq holds even chunk at partitions 0:64, odd at
            # 64:128 (cols = 128 h-cols of that chunk)
            c_tiles = [state.tile([P, P], f32, tag=f"c{q}", name=f"c{q}")
                       for q in range(NPAIR)]
            for k in range(NK):
                nc.sync.dma_start(out=hA[k][:], in_=hT0[k * P:(k + 1) * P, :])
            for q, ct in enumerate(c_tiles):
                nc.sync.dma_start(out=ct[:], in_=c0[:, q * P:(q + 1) * P])

            def mm_pair(G, hsrc, cq, ks):
                """Accumulate gate chunk pair cq over k-tiles `ks` into the
                merged PSUM tile G [128, 512]."""
                ce, co = 2 * cq, 2 * cq + 1
                last = (NK - 1) if bias_mode != "mm" else -1
                for k in ks:
                    nc.tensor.matmul(
                        G[0:B, :], lhsT=hsrc[k][:],
                        rhs=W_sb[:, k * G4 + ce * 512: k * G4 + ce * 512 + 512],
                        start=(k == 0), stop=(k == last),
                        tile_position=(0, 0), skip_group_check=True,
                    )
                    nc.tensor.matmul(
                        G[B:P, :], lhsT=hsrc[k][:],
                        rhs=W_sb[:, k * G4 + co * 512: k * G4 + co * 512 + 512],
                        start=(k == 0 and g1_start), stop=(k == last),
                        tile_position=(0, B), skip_group_check=True,
                    )
                if bias_mode == "mm" and (NK - 1) in ks:
                    nc.tensor.matmul(
                        G[0:P, :], lhsT=sel2[:],
                        rhs=bg2[:, cq * 512:(cq + 1) * 512],
                        start=False, stop=True, skip_group_check=True,
                    )

            def mm_pair_split(G0, G1, hsrc, cq, ks):
                ce, co = 2 * cq, 2 * cq + 1
                for k in ks:
                    nc.tensor.matmul(
                        G0[0:B, :], lhsT=hsrc[k][:],
                        rhs=W_sb[:, k * G4 + ce * 512: k * G4 + ce * 512 + 512],
                        start=(k == 0), stop=(k == NK - 1),
                        tile_position=(0, 0),
                    )
                    nc.tensor.matmul(
                        G1[B:P, :], lhsT=hsrc[k][:],
                        rhs=W_sb[:, k * G4 + co * 512: k * G4 + co * 512 + 512],
                        start=(k == 0), stop=(k == NK - 1),
                        tile_position=(0, B),
                    )

            def nl_merged(G, cq):
                """Bias + nonlinearities for pair cq, 128-partition ops.
                Returns the batch-major h pair tile (bf16)."""
                if bias_mode != "mm":
                    nc.vector.tensor_add(G[:, :], G[:, :],
                                         biasF[:, cq * 512:(cq + 1) * 512])
                sig = work.tile([P, 384], f32, tag="sig")
                nc.scalar.activation(sig[:], G[:, 0:384], AF.Sigmoid)
                tg = work.tile([P, P], f32, tag="tg")
                nc.scalar.activation(tg[:], G[:, 384:512], AF.Tanh)
                ct = c_tiles[cq]
                t1 = work.tile([P, P], f32, tag="t1")
                nc.vector.tensor_mul(t1[:], sig[:, 0:P], tg[:])
                t2 = work.tile([P, P], f32, tag="t2")
                nc.vector.tensor_mul(t2[:], sig[:, P:2 * P], ct[:])
                nc.vector.tensor_add(ct[:], t1[:], t2[:])
                tc2 = work.tile([P, P], f32, tag="tc2")
                nc.scalar.activation(tc2[:], ct[:], AF.Tanh)
                hbm2 = work.tile([P, P], bf16, tag="hbm")
                nc.vector.tensor_mul(hbm2[:], sig[:, 2 * P:3 * P], tc2[:])
                return hbm2

            def nl_split(G0, G1, cq):
                nc.vector.tensor_add(G0[0:B, :], G0[0:B, :],
                                     biasF[0:B, cq * 512:(cq + 1) * 512])
                nc.vector.tensor_add(G1[B:P, :], G1[B:P, :],
                                     biasF[B:P, cq * 512:(cq + 1) * 512])
                sig = work.tile([P, 384], f32, tag="sig")
                nc.scalar.activation(sig[0:B, :], G0[0:B, 0:384], AF.Sigmoid)
                nc.scalar.activation(sig[B:P, :], G1[B:P, 0:384], AF.Sigmoid)
                tg = work.tile([P, P], f32, tag="tg")
                nc.scalar.activation(tg[0:B, :], G0[0:B, 384:512], AF.Tanh)
                nc.scalar.activation(tg[B:P, :], G1[B:P, 384:512], AF.Tanh)
                ct = c_tiles[cq]
                t1 = work.tile([P, P], f32, tag="t1")
                nc.vector.tensor_mul(t1[:], sig[:, 0:P], tg[:])
                t2 = work.tile([P, P], f32, tag="t2")
                nc.vector.tensor_mul(t2[:], sig[:, P:2 * P], ct[:])
                nc.vector.tensor_add(ct[:], t1[:], t2[:])
                tc2 = work.tile([P, P], f32, tag="tc2")
                nc.scalar.activation(tc2[:], ct[:], AF.Tanh)
                hbm2 = work.tile([P, P], bf16, tag="hbm")
                nc.vector.tensor_mul(hbm2[:], sig[:, 2 * P:3 * P], tc2[:])
                return hbm2

            def tr(hbm2, hdst, cq, t_expr):
                """PE transpose of the pair's batch-major h; writes the two
                h_T chunks and streams them to the HBM history."""
                ce, co = 2 * cq, 2 * cq + 1
                pt = psumS.tile([P, P], bf16, tag="pt")
                nc.tensor.transpose(pt[:], hbm2[:], ident128[:])
                nc.vector.tensor_copy(hdst[ce][:], pt[:, 0:B])
                nc.vector.tensor_copy(hdst[co][:], pt[:, B:P])
                if not no_hstore:
                    nc.sync.dma_start(
                        out=hT_store[ce * P:(ce + 1) * P, ts(t_expr, B)],
                        in_=hdst[ce][:])
                    nc.sync.dma_start(
                        out=hT_store[co * P:(co + 1) * P, ts(t_expr, B)],
                        in_=hdst[co][:])

            def step(hsrc, hdst, t_expr, pend):
                """One decoder step; `pend` is the previous step's pair-3
                (hbm2, hdst, cq, t) transpose, emitted inside this step's
                first matmul block. Returns this step's pair-3 pend."""
                pends = [None] * NPAIR
                for cq in range(NPAIR):
                    if split_g:
                        Ga = psumG.tile([P, 512], f32, tag="Ga")
                        Gb = psumG.tile([P, 512], f32, tag="Gb")
                        if cq == 0 and pend is not None:
                            mm_pair_split(Ga, Gb, hsrc, cq, range(0, 6))
                            tr(*pend)
                            mm_pair_split(Ga, Gb, hsrc, cq, range(6, 8))
                        else:
                            mm_pair_split(Ga, Gb, hsrc, cq, range(NK))
                        if cq > 0:
                            tr(*pends[cq - 1])
                        pends[cq] = (nl_split(Ga, Gb, cq), hdst, cq, t_expr)
                    else:
                        G = psumG.tile([P, 512], f32, tag="G")
                        if cq == 0 and pend is not None:
                            mm_pair(G, hsrc, cq, range(0, 6))
                            tr(*pend)
                            mm_pair(G, hsrc, cq, range(6, 8))
                        else:
                            mm_pair(G, hsrc, cq, range(NK))
                        if cq > 0:
                            tr(*pends[cq - 1])
                        pends[cq] = (nl_merged(G, cq), hdst, cq, t_expr)
                return pends[NPAIR - 1]

            assert n_steps % unroll == 0 and unroll % 2 == 0
            with tc.For_i(0, n_steps // unroll, staggered_reset=True) as i:
                pend = None
                for j in range(unroll):
                    src, dst = (hA, hB) if j % 2 == 0 else (hB, hA)
                    pend = step(src, dst, i * unroll + j, pend)
                tr(*pend)

            # ---- deferred fc: out.T = Wf.T @ h_T + b_fc, T-sharded ----
            # Each core handles n_steps/8 steps = NK col-chunks of 512.
            pid_sp = nc.sync.partition_id()
            pid_act = nc.scalar.partition_id()
            nch_per_core = (n_steps * B) // 512 // 8
            for nch in range(nch_per_core):
                rhs = rpool.tile([P, NK * 512], bf16, tag="rhs")
                for k in range(NK):
                    # alternate issue queues: SP and ACT are both mostly
                    # idle in this tail phase; serial issue on one queue
                    # costs ~565ns per DMA
                    eng, pid = ((nc.sync, pid_sp) if k % 2 == 0
                                else (nc.scalar, pid_act))
                    eng.dma_start(
                        out=rhs[:, k * 512:(k + 1) * 512],
                        in_=hT_store[k * P:(k + 1) * P,
                                     ts(pid * nch_per_core + nch, 512)],
                    )
                for m in range(2):
                    O = psumS.tile([P, 512], f32, tag="O")
                    for k in range(NK):
                        nc.tensor.matmul(
                            O[:], lhsT=Wf_sb[:, k * L + m * P: k * L + (m + 1) * P],
                            rhs=rhs[:, k * 512:(k + 1) * 512],
                            start=(k == 0), stop=(k == NK - 1),
                        )
                    osb = work.tile([P, 512], f32, tag="osb")
                    nc.scalar.activation(osb[:], O[:], AF.Identity,
                                         bias=bfc_sb[:, m:m + 1])
                    nc.scalar.dma_start(
                        out=outsT[m * P:(m + 1) * P,
                                  ts(pid_act * nch_per_core + nch, 512)],
                        in_=osb[:])

    if not nc.is_finalized():
        nc.finalize()
    return nc


def _build_program_fp8p(mm_dt_name: str = "fp8p", n_steps: int = T):
    """fp8 DoubleRow gates with PAIR-MERGED PSUM banks: even chunk at
    partitions 0:64 (tile_position (0,0)), odd at 64:128 ((0,64)) — probing
    whether DoubleRow composes with a col-tile position on this HW. If it
    does, the whole nonlinearity runs at full 128-partition width again."""
    import concourse.bass as bass
    import concourse.bacc as bacc
    import concourse.mybir as mybir
    from concourse.bass import ts
    from concourse.tile import TileContext

    f32 = mybir.dt.float32
    bf16 = mybir.dt.bfloat16
    f8 = mybir.dt.float8e4
    DR = mybir.MatmulPerfMode.DoubleRow
    workbufs = int(os.environ.get("BASS_LSTM_WORKBUFS", "4"))
    psumg_bufs = int(os.environ.get("BASS_LSTM_PSUMG_BUFS", "3"))
    unroll = int(os.environ.get("BASS_LSTM_UNROLL", "32"))
    NJ = 4

    nc = bacc.Bacc(None, target_bir_lowering=False)
    AF = mybir.ActivationFunctionType

    hT0 = nc.declare_dram_parameter("hT0", [H, B], bf16, isOutput=False)
    c0 = nc.declare_dram_parameter("c0", [P, H // 2], f32, isOutput=False)
    Wg8 = nc.declare_dram_parameter("Wg8", [P, NCH * NJ * 1024], f8,
                                    isOutput=False)
    ones_d = nc.declare_dram_parameter("ones", [1, B], bf16, isOutput=False)
    bg_d = nc.declare_dram_parameter("bg", [1, G4], bf16, isOutput=False)
    Wf = nc.declare_dram_parameter("Wf", [H, L], bf16, isOutput=False)
    bfc_d = nc.declare_dram_parameter("bfc", [P, 2], f32, isOutput=False)
    ident_d = nc.declare_dram_parameter("ident128", [P, P], bf16,
                                        isOutput=False)
    hT_store = nc.dram_tensor("hT_store", [H, n_steps * B], bf16,
                              kind="Internal")
    outsT = nc.declare_dram_parameter("outsT", [L, n_steps * B], f32,
                                      isOutput=True)

    with TileContext(nc) as tc:
        with (
            tc.tile_pool(name="consts", bufs=1) as consts,
            tc.tile_pool(name="state", bufs=1) as state,
            tc.tile_pool(name="work", bufs=workbufs) as work,
            tc.tile_pool(name="rpool", bufs=2) as rpool,
            tc.tile_pool(name="psumG", bufs=psumg_bufs, space="PSUM") as psumG,
            tc.tile_pool(name="psumS", bufs=2, space="PSUM") as psumS,
        ):
            W8 = consts.tile([P, NCH * NJ, 2, 512], f8, tag="W8")
            nc.sync.dma_start(out=W8[:, :, :, :], in_=Wg8[:])
            ones_row = consts.tile([1, B], bf16, tag="ones")
            nc.sync.dma_start(out=ones_row[:], in_=ones_d[:])
            bg_sb = consts.tile([1, G4], bf16, tag="bg")
            nc.sync.dma_start(out=bg_sb[:], in_=bg_d[:])
            Wf_sb = consts.tile([P, NK * L], bf16, tag="Wf")
            for k in range(NK):
                nc.sync.dma_start(out=Wf_sb[:, k * L:(k + 1) * L],
                                  in_=Wf[k * P:(k + 1) * P, :])
            bfc_sb = consts.tile([P, 2], f32, tag="bfc")
            nc.sync.dma_start(out=bfc_sb[:], in_=bfc_d[:])
            ident128 = consts.tile([P, P], bf16, tag="ident128")
            nc.sync.dma_start(out=ident128[:], in_=ident_d[:])

            hpA = [state.tile([P, 2, B], f8, tag=f"hpA{j}", name=f"hpA{j}")
                   for j in range(NJ)]
            hpB = [state.tile([P, 2, B], f8, tag=f"hpB{j}", name=f"hpB{j}")
                   for j in range(NJ)]
            hds = [state.tile([P, B], bf16, tag=f"hd{k}", name=f"hd{k}")
                   for k in range(NK)]
            c_tiles = [state.tile([P, P], f32, tag=f"c{q}", name=f"c{q}")
                       for q in range(NPAIR)]
            h0sb = consts.tile([P, NK * B], bf16, tag="h0sb")
            for k in range(NK):
                nc.sync.dma_start(out=h0sb[:, k * B:(k + 1) * B],
                                  in_=hT0[k * P:(k + 1) * P, :])
            for j in range(NJ):
                nc.vector.tensor_copy(hpA[j][:, 0, :], h0sb[:, (2 * j) * B:(2 * j + 1) * B])
                nc.vector.tensor_copy(hpA[j][:, 1, :], h0sb[:, (2 * j + 1) * B:(2 * j + 2) * B])
            for q, ct in enumerate(c_tiles):
                nc.sync.dma_start(out=ct[:], in_=c0[:, q * P:(q + 1) * P])

            def mm_pair(G, hp_src, cq, js):
                """Bias (h-independent, start=True) then DoubleRow k-tiles
                for both chunks of pair cq into the merged bank."""
                ce, co = 2 * cq, 2 * cq + 1
                if js[0] == 0:
                    nc.tensor.matmul(
                        G[0:B, :], lhsT=ones_row[:],
                        rhs=bg_sb[:, ce * 512:(ce + 1) * 512],
                        start=True, stop=False, tile_position=(0, 0),
                        skip_group_check=True,
                    )
                    nc.tensor.matmul(
                        G[B:P, :], lhsT=ones_row[:],
                        rhs=bg_sb[:, co * 512:(co + 1) * 512],
                        start=True, stop=False, tile_position=(0, B),
                        skip_group_check=True,
                    )
                for j in js:
                    nc.tensor.matmul(
                        G[0:B, :], lhsT=hp_src[j][:, :, :],
                        rhs=W8[:, ce * NJ + j, :, :],
                        start=False, stop=(j == NJ - 1),
                        tile_position=(0, 0),
                        perf_mode=DR, skip_group_check=True,
                    )
                    nc.tensor.matmul(
                        G[B:P, :], lhsT=hp_src[j][:, :, :],
                        rhs=W8[:, co * NJ + j, :, :],
                        start=False, stop=(j == NJ - 1),
                        tile_position=(0, B),
                        perf_mode=DR, skip_group_check=True,
                    )

            def nl_merged(G, cq):
                sig = work.tile([P, 384], f32, tag="sig")
                nc.scalar.activation(sig[:], G[:, 0:384], AF.Sigmoid)
                tg = work.tile([P, P], f32, tag="tg")
                nc.scalar.activation(tg[:], G[:, 384:512], AF.Tanh)
                ct = c_tiles[cq]
                t1 = work.tile([P, P], f32, tag="t1")
                nc.vector.tensor_mul(t1[:], sig[:, 0:P], tg[:])
                t2 = work.tile([P, P], f32, tag="t2")
                nc.vector.tensor_mul(t2[:], sig[:, P:2 * P], ct[:])
                nc.vector.tensor_add(ct[:], t1[:], t2[:])
                tc2 = work.tile([P, P], f32, tag="tc2")
                nc.scalar.activation(tc2[:], ct[:], AF.Tanh)
                hbm2 = work.tile([P, P], bf16, tag="hbm")
                nc.vector.tensor_mul(hbm2[:], sig[:, 2 * P:3 * P], tc2[:])
                return hbm2

            def tr(hbm2, hp_dst, cq, t_expr):
                ce, co = 2 * cq, 2 * cq + 1
                pt = psumS.tile([P, P], bf16, tag="pt")
                nc.tensor.transpose(pt[:], hbm2[:], ident128[:])
                nc.vector.tensor_copy(hds[ce][:], pt[:, 0:B])
                nc.vector.tensor_copy(hds[co][:], pt[:, B:P])
                # one full-width fp8 cast rebuilds the whole stationary pair
                nc.vector.tensor_copy(hp_dst[cq][:, :, :], pt[:, 0:P])
                nc.sync.dma_start(
                    out=hT_store[ce * P:(ce + 1) * P, ts(t_expr, B)],
                    in_=hds[ce][:])
                nc.sync.dma_start(
                    out=hT_store[co * P:(co + 1) * P, ts(t_expr, B)],
                    in_=hds[co][:])

            def step(hp_src, hp_dst, t_expr, pend):
                pends = [None] * NPAIR
                for cq in range(NPAIR):
                    G = psumG.tile([P, 512], f32, tag="G")
                    if cq == 0 and pend is not None:
                        mm_pair(G, hp_src, cq, [0, 1, 2])
                        tr(*pend)
                        mm_pair(G, hp_src, cq, [3])
                    else:
                        mm_pair(G, hp_src, cq, [0, 1, 2, 3])
                    if cq > 0:
                        tr(*pends[cq - 1])
                    pends[cq] = (nl_merged(G, cq), hp_dst, cq, t_expr)
                return pends[NPAIR - 1]

            assert n_steps % unroll == 0 and unroll % 2 == 0
            with tc.For_i(0, n_steps // unroll, staggered_reset=True) as i:
                pend = None
                for j in range(unroll):
                    src, dst = (hpA, hpB) if j % 2 == 0 else (hpB, hpA)
                    pend = step(src, dst, i * unroll + j, pend)
                tr(*pend)

            pid_sp = nc.sync.partition_id()
            pid_act = nc.scalar.partition_id()
            nch_per_core = (n_steps * B) // 512 // 8
            for nch in range(nch_per_core):
                rhs = rpool.tile([P, NK * 512], bf16, tag="rhs")
                for k in range(NK):
                    # alternate issue queues: SP and ACT are both mostly
                    # idle in this tail phase; serial issue on one queue
                    # costs ~565ns per DMA
                    eng, pid = ((nc.sync, pid_sp) if k % 2 == 0
                                else (nc.scalar, pid_act))
                    eng.dma_start(
                        out=rhs[:, k * 512:(k + 1) * 512],
                        in_=hT_store[k * P:(k + 1) * P,
                                     ts(pid * nch_per_core + nch, 512)],
                    )
                for m in range(2):
                    O = psumS.tile([P, 512], f32, tag="O")
                    for k in range(NK):
                        nc.tensor.matmul(
                            O[:], lhsT=Wf_sb[:, k * L + m * P: k * L + (m + 1) * P],
                            rhs=rhs[:, k * 512:(k + 1) * 512],
                            start=(k == 0), stop=(k == NK - 1),
                        )
                    osb = work.tile([P, 512], f32, tag="osb")
                    nc.scalar.activation(osb[:], O[:], AF.Identity,
                                         bias=bfc_sb[:, m:m + 1])
                    nc.scalar.dma_start(
                        out=outsT[m * P:(m + 1) * P,
                                  ts(pid_act * nch_per_core + nch, 512)],
                        in_=osb[:])

    if not nc.is_finalized():
        nc.finalize()
    return nc


def _build_program_fp8(mm_dt_name: str = "fp8dr", n_steps: int = T):
    """fp8e4m3 DoubleRow gate matmuls: K=256 per matmul (2 fp8 weights per
    PE cell) -> 32 gate matmuls/step instead of 64. Gates land [64,512] at
    partitions 0:64 (DoubleRow excludes col tiling), so the nonlinearity
    runs per-chunk; h pairs are reassembled into [128,2,64] fp8 stationary
    tiles. h history / fc GEMM stay bf16 for output accuracy."""
    import concourse.bass as bass
    import concourse.bacc as bacc
    import concourse.mybir as mybir
    from concourse.bass import ts
    from concourse.tile import TileContext

    f32 = mybir.dt.float32
    bf16 = mybir.dt.bfloat16
    f8 = mybir.dt.float8e4
    DR = mybir.MatmulPerfMode.DoubleRow
    workbufs = int(os.environ.get("BASS_LSTM_WORKBUFS", "4"))
    psumg_bufs = int(os.environ.get("BASS_LSTM_PSUMG_BUFS", "4"))
    unroll = int(os.environ.get("BASS_LSTM_UNROLL", "64"))
    NJ = 4          # K=256 tiles
    ROT = 1         # cross-step transpose deferral depth (1 = last chunk)

    nc = bacc.Bacc(None, target_bir_lowering=False)
    AF = mybir.ActivationFunctionType

    hT0 = nc.declare_dram_parameter("hT0", [H, B], bf16, isOutput=False)
    c0 = nc.declare_dram_parameter("c0", [B, H], f32, isOutput=False)
    # fp8 W: block b = ch*NJ + j holds [128, 2, 512]:
    #   Wg8[p, b, i, n] = W_eff_perm[(2j+i)*128 + p, ch*512 + n]
    Wg8 = nc.declare_dram_parameter("Wg8", [P, NCH * NJ * 1024], f8,
                                    isOutput=False)
    ones_d = nc.declare_dram_parameter("ones", [1, B], bf16, isOutput=False)
    bg_d = nc.declare_dram_parameter("bg", [1, G4], bf16, isOutput=False)
    Wf = nc.declare_dram_parameter("Wf", [H, L], bf16, isOutput=False)
    bfc_d = nc.declare_dram_parameter("bfc", [P, 2], f32, isOutput=False)
    ident64_d = nc.declare_dram_parameter("ident64", [B, B], bf16,
                                          isOutput=False)
    hT_store = nc.dram_tensor("hT_store", [H, n_steps * B], bf16,
                              kind="Internal")
    outsT = nc.declare_dram_parameter("outsT", [L, n_steps * B], f32,
                                      isOutput=True)

    with TileContext(nc) as tc:
        with (
            tc.tile_pool(name="consts", bufs=1) as consts,
            tc.tile_pool(name="state", bufs=1) as state,
            tc.tile_pool(name="work", bufs=workbufs) as work,
            tc.tile_pool(name="rpool", bufs=2) as rpool,
            tc.tile_pool(name="psumG", bufs=psumg_bufs, space="PSUM") as psumG,
            tc.tile_pool(name="psumS", bufs=2, space="PSUM") as psumS,
        ):
            W8 = consts.tile([P, NCH * NJ, 2, 512], f8, tag="W8")
            nc.sync.dma_start(out=W8[:, :, :, :], in_=Wg8[:])
            ones_row = consts.tile([1, B], bf16, tag="ones")
            nc.sync.dma_start(out=ones_row[:], in_=ones_d[:])
            bg_sb = consts.tile([1, G4], bf16, tag="bg")
            nc.sync.dma_start(out=bg_sb[:], in_=bg_d[:])
            Wf_sb = consts.tile([P, NK * L], bf16, tag="Wf")
            for k in range(NK):
                nc.sync.dma_start(out=Wf_sb[:, k * L:(k + 1) * L],
                                  in_=Wf[k * P:(k + 1) * P, :])
            bfc_sb = consts.tile([P, 2], f32, tag="bfc")
            nc.sync.dma_start(out=bfc_sb[:], in_=bfc_d[:])
            ident64 = consts.tile([B, B], bf16, tag="ident64")
            nc.sync.dma_start(out=ident64[:], in_=ident64_d[:])

            # fp8 stationary pairs: hp[j] [128, 2, 64]; bf16 h_T chunks for
            # the HBM history
            hpA = [state.tile([P, 2, B], f8, tag=f"hpA{j}", name=f"hpA{j}")
                   for j in range(NJ)]
            hpB = [state.tile([P, 2, B], f8, tag=f"hpB{j}", name=f"hpB{j}")
                   for j in range(NJ)]
            hds = [state.tile([P, B], bf16, tag=f"hd{k}", name=f"hd{k}")
                   for k in range(NK)]
            c_tiles = [state.tile([B, P], f32, tag=f"c{k}", name=f"c{k}")
                       for k in range(NCH)]
            # initial state: hT0 -> both fp8 pair tiles (via bf16 staging)
            h0sb = consts.tile([P, NK * B], bf16, tag="h0sb")
            for k in range(NK):
                nc.sync.dma_start(out=h0sb[:, k * B:(k + 1) * B],
                                  in_=hT0[k * P:(k + 1) * P, :])
            for j in range(NJ):
                nc.vector.tensor_copy(hpA[j][:, 0, :], h0sb[:, (2 * j) * B:(2 * j + 1) * B])
                nc.vector.tensor_copy(hpA[j][:, 1, :], h0sb[:, (2 * j + 1) * B:(2 * j + 2) * B])
            for k in range(NCH):
                nc.sync.dma_start(out=c_tiles[k][:], in_=c0[:, k * P:(k + 1) * P])

            def mm_block(G, hp_src, ch, js):
                """Bias matmul (h-independent, start=True) then DoubleRow
                k-tile accumulation for gate chunk ch."""
                if js[0] == 0:
                    nc.tensor.matmul(
                        G[0:B, :], lhsT=ones_row[:],
                        rhs=bg_sb[:, ch * 512:(ch + 1) * 512],
                        start=True, stop=False, skip_group_check=True,
                    )
                for j in js:
                    nc.tensor.matmul(
                        G[0:B, :], lhsT=hp_src[j][:, :, :],
                        rhs=W8[:, ch * NJ + j, :, :],
                        start=False, stop=(j == NJ - 1),
                        perf_mode=DR, skip_group_check=True,
                    )

            def nl_a1(G):
                """ACT-only front stage: sigmoid + tanh(g)."""
                sig = work.tile([B, 384], f32, tag="sig")
                nc.scalar.activation(sig[:], G[0:B, 0:384], AF.Sigmoid)
                tg = work.tile([B, P], f32, tag="tg")
                nc.scalar.activation(tg[:], G[0:B, 384:512], AF.Tanh)
                return sig, tg

            def nl_d1(sig, tg, ch):
                """c update (DVE): c = sig_f*c + sig_i*tanh_g."""
                ct = c_tiles[ch]
                t1 = work.tile([B, P], f32, tag="t1")
                nc.vector.tensor_mul(t1[:], sig[:, 0:P], tg[:])
                t2 = work.tile([B, P], f32, tag="t2")
                nc.vector.tensor_mul(t2[:], sig[:, P:2 * P], ct[:])
                nc.vector.tensor_add(ct[:], t1[:], t2[:])

            def nl_a2(ch):
                tc2 = work.tile([B, P], f32, tag="tc2")
                nc.scalar.activation(tc2[:], c_tiles[ch][:], AF.Tanh)
                return tc2

            def nl_d2(sig, tc2):
                hbm = work.tile([B, P], bf16, tag="hbm")
                nc.vector.tensor_mul(hbm[:], sig[:, 2 * P:3 * P], tc2[:])
                return hbm

            def tr(hbm, hp_dst, ch, t_expr):
                pt = psumS.tile([P, B], bf16, tag="pt")
                nc.tensor.transpose(pt[:], hbm[0:B, :], ident64[:])
                nc.vector.tensor_copy(hds[ch][:], pt[:])
                # fp8 stationary slot rebuilt from the bf16 SBUF copy (SBUF
                # read is cheaper than PSUM, and Pool cannot touch PSUM)
                nc.gpsimd.tensor_copy(hp_dst[ch // 2][:, ch % 2, :], hds[ch][:])
                nc.sync.dma_start(
                    out=hT_store[ch * P:(ch + 1) * P, ts(t_expr, B)],
                    in_=hds[ch][:])

            def drain(pend):
                """Finish the previous step's chunk 5/6/7 stages. Emitted
                either inside the next step's block 0 (split around the
                matmul j-tiles that consume the rebuilt hp pairs) or at the
                iteration tail."""
                hbm5, (sig6, tc26), (sig7, tg7), hpd, tp = pend
                nl_d1(sig7, tg7, 7)
                tc27 = nl_a2(7)
                hbm6 = nl_d2(sig6, tc26)
                tr(hbm5, hpd, 5, tp)
                hbm7 = nl_d2(sig7, tc27)
                tr(hbm6, hpd, 6, tp)
                return (hbm7, hpd, tp)

            def step(hp_src, hp_dst, t_expr, pend):
                """Deep software pipeline; emission per block ch:
                MM(ch) -> A1(ch) -> D1(ch-1) -> A2(ch-1) -> D2(ch-2) ->
                TR(ch-3).  Neither ACT nor DVE FIFO ever waits on the other
                engine's round-trip."""
                a1 = {}    # ch -> (sig, tg)
                tc2s = {}  # ch -> tc2 tile
                hbms = {}  # ch -> hbm tile
                for ch in range(NCH):
                    G = psumG.tile([P, 512], f32, tag="G")
                    if ch == 0 and pend is not None:
                        mm_block(G, hp_src, ch, [0, 1])
                        hbm7, hpd, tp = drain(pend)
                        mm_block(G, hp_src, ch, [2])
                        tr(hbm7, hpd, 7, tp)
                        mm_block(G, hp_src, ch, [3])
                    else:
                        mm_block(G, hp_src, ch, [0, 1, 2, 3])
                    a1[ch] = nl_a1(G)
                    if ch - 1 >= 0:
                        nl_d1(*a1[ch - 1], ch - 1)
                        tc2s[ch - 1] = nl_a2(ch - 1)
                    if ch - 2 >= 0:
                        hbms[ch - 2] = nl_d2(a1[ch - 2][0], tc2s[ch - 2])
                    if ch - 3 >= 0:
                        tr(hbms[ch - 3], hp_dst, ch - 3, t_expr)
                return (hbms[5], (a1[6][0], tc2s[6]), a1[7], hp_dst, t_expr)

            assert n_steps % unroll == 0 and unroll % 2 == 0
            with tc.For_i(0, n_steps // unroll, staggered_reset=True) as i:
                pend = None
                for j in range(unroll):
                    src, dst = (hpA, hpB) if j % 2 == 0 else (hpB, hpA)
                    pend = step(src, dst, i * unroll + j, pend)
                hbm7, hpd, tp = drain(pend)
                tr(hbm7, hpd, 7, tp)

            pid_sp = nc.sync.partition_id()
            pid_act = nc.scalar.partition_id()
            nch_per_core = (n_steps * B) // 512 // 8
            for nch in range(nch_per_core):
                rhs = rpool.tile([P, NK * 512], bf16, tag="rhs")
                for k in range(NK):
                    # alternate issue queues: SP and ACT are both mostly
                    # idle in this tail phase; serial issue on one queue
                    # costs ~565ns per DMA
                    eng, pid = ((nc.sync, pid_sp) if k % 2 == 0
                                else (nc.scalar, pid_act))
                    eng.dma_start(
                        out=rhs[:, k * 512:(k + 1) * 512],
                        in_=hT_store[k * P:(k + 1) * P,
                                     ts(pid * nch_per_core + nch, 512)],
                    )
                for m in range(2):
                    O = psumS.tile([P, 512], f32, tag="O")
                    for k in range(NK):
                        nc.tensor.matmul(
                            O[:], lhsT=Wf_sb[:, k * L + m * P: k * L + (m + 1) * P],
                            rhs=rhs[:, k * 512:(k + 1) * 512],
                            start=(k == 0), stop=(k == NK - 1),
                        )
                    osb = work.tile([P, 512], f32, tag="osb")
                    nc.scalar.activation(osb[:], O[:], AF.Identity,
                                         bias=bfc_sb[:, m:m + 1])
                    nc.scalar.dma_start(
                        out=outsT[m * P:(m + 1) * P,
                                  ts(pid_act * nch_per_core + nch, 512)],
                        in_=osb[:])

    if not nc.is_finalized():
        nc.finalize()
    return nc


def _prepare_host_inputs(initial_input, h0, c0, w_ih, w_hh, b_ih, b_hh,
                         w_fc, b_fc, mm_dt_name="bf16"):
    """Host: fp32 step 0 + collapsed weights, permuted for the device."""
    import ml_dtypes

    f64 = np.float64
    w_ih64, w_hh64 = w_ih.astype(f64), w_hh.astype(f64)
    w_fc64, b_fc64 = w_fc.astype(f64), b_fc.astype(f64)
    bias64 = b_ih.astype(f64) + b_hh.astype(f64)

    W_eff = (w_fc64.T @ w_ih64.T + w_hh64.T).astype(np.float32)   # [H, 4H]
    b_eff = (bias64 + b_fc64 @ w_ih64.T).astype(np.float32)       # [4H]

    def sigmoid(x):
        return 1.0 / (1.0 + np.exp(-x))

    x = initial_input.astype(np.float32)
    h = h0[0].astype(np.float32)
    c = c0[0].astype(np.float32)
    g = x @ w_ih.T.astype(np.float32) + h @ w_hh.T.astype(np.float32) \
        + (bias64.astype(np.float32))
    i_, f_, g_, o_ = np.split(g, 4, axis=1)
    c = sigmoid(f_) * c + sigmoid(i_) * np.tanh(g_)
    h = sigmoid(o_) * np.tanh(c)
    out0 = h @ w_fc.T.astype(np.float32) + b_fc.astype(np.float32)

    perm = _gate_perm()
    Wg = np.ascontiguousarray(W_eff[:, perm])
    bg = np.ascontiguousarray(b_eff[perm])            # [4096] permuted
    Wf = np.ascontiguousarray(w_fc.T.astype(np.float32))
    hT = np.ascontiguousarray(h.T)

    bf16 = ml_dtypes.bfloat16

    # biasF [128, NPAIR*512]: block cq rows 0:64 even-chunk bias, 64:128 odd
    biasF = np.empty((P, NPAIR * 512), np.float32)
    for cq in range(NPAIR):
        ce, co = 2 * cq, 2 * cq + 1
        biasF[0:B, cq * 512:(cq + 1) * 512] = bg[ce * 512:(ce + 1) * 512]
        biasF[B:P, cq * 512:(cq + 1) * 512] = bg[co * 512:(co + 1) * 512]

    # c packed [64, 1024] -> [128, 512]: chunk k at partitions (k%2)*64,
    # cols (k//2)*128
    c_packed = np.zeros((P, H // 2), np.float32)
    for k in range(NK):
        c_packed[(k % 2) * B:(k % 2) * B + B,
                 (k // 2) * P:(k // 2) * P + P] = c[:, k * P:(k + 1) * P]

    bfc = np.stack([b_fc[0:P], b_fc[P:2 * P]], axis=1).astype(np.float32)

    if mm_dt_name in ("fp8dr", "fp8p"):
        from concourse import mybir
        f8np = mybir.dt.np(mybir.dt.float8e4)
        NJ = 4
        # Wg8[p, (ch*NJ+j)*1024 + i*512 + n] = Wg[(2j+i)*128 + p, ch*512 + n]
        Wr = Wg.reshape(NK, P, NCH, 512)          # [k128, p, ch, n]
        Wr = Wr.reshape(NJ, 2, P, NCH, 512)       # [j, i, p, ch, n]
        Wg8 = np.ascontiguousarray(
            np.transpose(Wr, (2, 3, 0, 1, 4)).reshape(P, NCH * NJ * 1024))
        bfc = np.stack([b_fc[0:P], b_fc[P:2 * P]], axis=1).astype(np.float32)
        in_map = {
            "hT0": hT.astype(bf16),
            "Wg8": Wg8.astype(f8np),
            "ones": np.ones((1, B), np.float32).astype(bf16),
            "bg": bg[None, :].astype(bf16),
            "Wf": Wf.astype(bf16),
            "bfc": np.ascontiguousarray(bfc),
        }
        if mm_dt_name == "fp8p":
            c_packed = np.zeros((P, H // 2), np.float32)
            for k in range(NK):
                c_packed[(k % 2) * B:(k % 2) * B + B,
                         (k // 2) * P:(k // 2) * P + P] = c[:, k * P:(k + 1) * P]
            in_map["c0"] = np.ascontiguousarray(c_packed)
            in_map["ident128"] = np.eye(P, dtype=np.float32).astype(bf16)
        else:
            in_map["c0"] = c.astype(np.float32)
            in_map["ident64"] = np.eye(B, dtype=np.float32).astype(bf16)
        return in_map, out0

    sel2 = np.zeros((2, P), np.float32)
    sel2[0, 0:B] = 1.0
    sel2[1, B:P] = 1.0
    bg2 = np.empty((2, NPAIR * 512), np.float32)
    for cq in range(NPAIR):
        bg2[0, cq * 512:(cq + 1) * 512] = bg[(2 * cq) * 512:(2 * cq + 1) * 512]
        bg2[1, cq * 512:(cq + 1) * 512] = bg[(2 * cq + 1) * 512:(2 * cq + 2) * 512]

    in_map = {
        "hT0": hT.astype(bf16),
        "c0": np.ascontiguousarray(c_packed),
        "Wg": Wg.astype(bf16),
        "biasF": biasF,
        "sel2": sel2.astype(bf16),
        "bg2": bg2.astype(bf16),
        "Wf": Wf.astype(bf16),
        "bfc": np.ascontiguousarray(bfc),
        "ident128": np.eye(P, dtype=np.float32).astype(bf16),
    }
    return in_map, out0


LAST_EXEC_NS = None

# min over jax.random.uniform(jax.random.key(42), (512,)) — the per-step
# teacher-forcing draws inside the reference. tf_prob below this means the
# decoder is purely autoregressive (the case the device kernel implements).
_RAND_MIN = 5.8138370513916016e-04


def _kernel_numpy_fallback(initial_input, h0, c0, targets, tf_prob,
                           w_ih, w_hh, b_ih, b_hh, w_fc, b_fc):
    """Host fp32 implementation incl. teacher forcing (only used if
    tf_prob >= min(rand), which the problem spec never produces)."""
    import jax
    import jax.numpy as jnp
    cpu = jax.devices("cpu")[0]
    with jax.default_device(cpu):
        rand = np.asarray(
            jax.random.uniform(jax.random.key(42), (T,), jnp.float32))

    def sigmoid(x):
        return 1.0 / (1.0 + np.exp(-x))

    bias = (b_ih + b_hh).astype(np.float32)
    h = h0[0].astype(np.float32)
    c = c0[0].astype(np.float32)
    inp = initial_input.astype(np.float32)
    outs = []
    for t in range(T):
        g = inp @ w_ih.T + h @ w_hh.T + bias
        i, f, gg, o = np.split(g, 4, axis=1)
        c = sigmoid(f) * c + sigmoid(i) * np.tanh(gg)
        h = sigmoid(o) * np.tanh(c)
        out = h @ w_fc.T + b_fc
        inp = out if rand[t] > tf_prob else targets[:, t, :]
        outs.append(out)
    return np.stack(outs, axis=1)[:, None, :, :].astype(np.float32)


def _assemble_output(results, out0, n_cores=8):
    """Assemble [B, 1, T, L] from per-core outsT slices."""
    nch_per_core = (T * B) // 512 // n_cores
    cols_per_core = nch_per_core * 512
    outT = np.empty((L, T * B), np.float32)
    for c in range(n_cores):
        sl = slice(c * cols_per_core, (c + 1) * cols_per_core)
        outT[:, sl] = results[c]["outsT"][:, sl]
    # col t*B+b holds out for step t+1; steps 1..511 used
    o = outT.reshape(L, T, B)
    out = np.empty((B, 1, T, L), np.float32)
    out[:, 0, 0, :] = out0
    out[:, 0, 1:, :] = np.transpose(o[:, :T - 1, :], (2, 1, 0))
    return out


def kernel(initial_input, encoder_outputs, h0, c0, targets, tf_prob,
           w_ih, w_hh, b_ih, b_hh, w_fc, b_fc):
    global LAST_EXEC_NS
    from concourse.bass_utils import run_bass_kernel_spmd

    if float(np.asarray(tf_prob)) >= _RAND_MIN:
        return _kernel_numpy_fallback(
            np.asarray(initial_input), np.asarray(h0), np.asarray(c0),
            np.asarray(targets), float(np.asarray(tf_prob)),
            np.asarray(w_ih), np.asarray(w_hh), np.asarray(b_ih),
            np.asarray(b_hh), np.asarray(w_fc), np.asarray(b_fc))

    mm_dt_name = os.environ.get("BASS_LSTM_DT", "fp8dr")
    n_cores = 8

    if mm_dt_name not in _prog_cache:
        _prog_cache[mm_dt_name] = _build_program(mm_dt_name)
    nc = _prog_cache[mm_dt_name]

    in_map, out0 = _prepare_host_inputs(
        np.asarray(initial_input), np.asarray(h0), np.asarray(c0),
        np.asarray(w_ih), np.asarray(w_hh), np.asarray(b_ih),
        np.asarray(b_hh), np.asarray(w_fc), np.asarray(b_fc), mm_dt_name,
    )

    core_ids = list(range(n_cores))
    res = run_bass_kernel_spmd(nc, [in_map] * n_cores, core_ids=core_ids)
    LAST_EXEC_NS = res.exec_time_ns
    return _assemble_output(res.results, out0, n_cores)
